# revision 6
# baseline (speedup 1.0000x reference)
"""Trainium2 Bass kernel for nn_BinarizedLinear:
    out = sign(input_b @ sign(weight).T)
with input_b (8192, 2048) and weight (2048, 2048), entries all +/-1.0 fp32.

Since weight entries are +/-1, sign(weight) == weight; the linear output is a
sum of 2048 +/-1 terms, i.e. an even integer in [-2048, 2048], so
sign(v) == clamp(v, -1, 1) exactly and bf16 operands are bit-exact.

Strategy: data-parallel across 8 NeuronCores — each core gets 1024 rows of
input_b, the full weight replicated.  Per core:
  - DMA fp32 tiles in, cast to bf16 on GpSimd (exact for +/-1),
  - transpose x and W tiles on the TensorEngine (contraction dim must live on
    SBUF partitions for matmul); 8 transposes pack into one PSUM bank so each
    DVE/ACT eviction moves 1024 columns,
  - matmul bf16 (full PE rate), accumulating k=2048 into PSUM fp32,
  - fuse sign() into the PSUM->SBUF eviction as a single tensor_scalar
    (min 1.0 then max -1.0), DMA the fp32 result out.
Emission order pipelines DMA ahead of the PE: W-block 0 loads first, x-shard
transposes stay one b-tile ahead of the matmuls, and the next W block's
transposes are spread between matmul blocks.
"""

import numpy as np

BATCH, IN_LEN, OUT_LEN = 8192, 2048, 2048
N_CORES = 8
SHARD = BATCH // N_CORES  # 1024
P = 128

_cache = {}


def build_kernel(shard=SHARD, in_len=IN_LEN, out_len=OUT_LEN):
    import concourse.mybir as mybir
    import concourse.tile as tile
    from concourse import bacc
    from concourse.masks import make_identity

    f32 = mybir.dt.float32
    bf16 = mybir.dt.bfloat16

    KT = in_len // P          # k-tiles (contraction)
    BT = shard // P           # b-tiles per core
    OB = out_len // 512       # 512-wide output blocks
    OJ = 512 // P             # 128-row W chunks per o-block
    KP = min(8, KT)           # transposes packed per PSUM bank eviction

    nc = bacc.Bacc(None, target_bir_lowering=False)
    x = nc.dram_tensor("x", [shard, in_len], f32, kind="ExternalInput")
    w = nc.dram_tensor("w", [out_len, in_len], f32, kind="ExternalInput")
    out = nc.dram_tensor("out", [shard, out_len], f32, kind="ExternalOutput")

    with tile.TileContext(nc) as tc:
        with (
            tc.tile_pool(name="const", bufs=1) as const_pool,
            tc.tile_pool(name="xt", bufs=BT) as xt_pool,
            tc.tile_pool(name="stage", bufs=3) as stage_pool,
            tc.tile_pool(name="bstage", bufs=3) as bstage_pool,
            tc.tile_pool(name="wtblk", bufs=2) as wt_pool,
            tc.tile_pool(name="outs", bufs=4) as out_pool,
            tc.tile_pool(name="tpsum", bufs=3, space="PSUM") as tpsum_pool,
            tc.tile_pool(name="mpsum", bufs=4, space="PSUM") as mpsum_pool,
        ):
            ident = const_pool.tile([P, P], bf16)
            make_identity(nc, ident)

            def load_cast(dram, row0):
                """DMA a [128, in_len] fp32 slab, cast to bf16 on GpSimd."""
                stage = stage_pool.tile([P, in_len], f32, tag="stage")
                nc.sync.dma_start(out=stage[:], in_=dram[row0:row0 + P, :])
                bst = bstage_pool.tile([P, in_len], bf16, tag="bstage")
                nc.gpsimd.tensor_copy(out=bst[:], in_=stage[:])
                return bst

            def transpose_into(bst, dest_fn):
                """PE-transpose [128,128] sub-tiles of bst; pack KP per PSUM
                bank, then evict each bank with one wide copy via dest_fn."""
                for k0 in range(0, KT, KP):
                    tp = tpsum_pool.tile([P, KP * P], bf16, tag="tp")
                    for q in range(KP):
                        nc.tensor.transpose(
                            tp[:, q * P:(q + 1) * P],
                            bst[:, (k0 + q) * P:(k0 + q + 1) * P],
                            ident[:],
                        )
                    dest_fn(k0, tp)

            # xt[bt][p, k, b] = x[bt*128 + b, k*128 + p]  (bf16)
            xt = {}

            def emit_x(bt):
                bst = load_cast(x, bt * P)
                xt[bt] = xt_pool.tile([P, KT, P], bf16, tag="xt", name=f"xt{bt}")

                def dest(k0, tp):
                    nc.any.tensor_copy(
                        out=xt[bt][:, k0:k0 + KP, :],
                        in_=tp[:].rearrange("p (k b) -> p k b", k=KP),
                    )

                transpose_into(bst, dest)

            # wt[p, k, j*128+o] = w[ob*512 + j*128 + o, k*128 + p]  (bf16)
            def emit_w_chunk(wt_blk, ob, j):
                bst = load_cast(w, (ob * OJ + j) * P)

                def dest(k0, tp):
                    nc.any.tensor_copy(
                        out=wt_blk[:, k0:k0 + KP, j * P:(j + 1) * P],
                        in_=tp[:].rearrange("p (k b) -> p k b", k=KP),
                    )

                transpose_into(bst, dest)

            def emit_mm(wt_blk, ob, bt):
                psum = mpsum_pool.tile([P, 512], f32)
                for k in range(KT):
                    nc.tensor.matmul(
                        psum[:],
                        xt[bt][:, k, :],
                        wt_blk[:, k, :],
                        start=(k == 0),
                        stop=(k == KT - 1),
                    )
                ot = out_pool.tile([P, 512], f32)
                # sign(v) for integer v: clamp to [-1, 1]
                nc.any.tensor_scalar(
                    out=ot[:], in0=psum[:], scalar1=1.0, scalar2=-1.0,
                    op0=mybir.AluOpType.min, op1=mybir.AluOpType.max,
                )
                nc.sync.dma_start(
                    out=out[bt * P:(bt + 1) * P, ob * 512:(ob + 1) * 512],
                    in_=ot[:],
                )

            wt_blks = {}
            # W block 0 first so its DMAs lead the queue
            wt_blks[0] = wt_pool.tile([P, KT, 512], bf16, tag="wtblk", name="wt0")
            for j in range(OJ):
                emit_w_chunk(wt_blks[0], 0, j)
            # x transposes stay one b-tile ahead of the matmuls
            emit_x(0)
            for ob in range(OB):
                if ob + 1 < OB:
                    wt_blks[ob + 1] = wt_pool.tile(
                        [P, KT, 512], bf16, tag="wtblk", name=f"wt{ob + 1}"
                    )
                emitted_j = 0
                for bt in range(BT):
                    if ob == 0 and bt + 1 < BT:
                        emit_x(bt + 1)
                    emit_mm(wt_blks[ob], ob, bt)
                    # spread next W block's chunks between matmul blocks
                    if ob + 1 < OB:
                        j_target = -(-OJ * (bt + 1) // BT)  # ceil
                        while emitted_j < j_target:
                            emit_w_chunk(wt_blks[ob + 1], ob + 1, emitted_j)
                            emitted_j += 1

    nc.finalize()
    return nc


def _get_nc():
    if "nc" not in _cache:
        _cache["nc"] = build_kernel()
    return _cache["nc"]


def run_sharded(input_b, weight, trace=False):
    """Run the SPMD kernel; returns (output, BassKernelResults)."""
    from concourse.bass_utils import run_bass_kernel_spmd

    nc = _get_nc()
    input_b = np.ascontiguousarray(input_b, dtype=np.float32)
    weight = np.ascontiguousarray(weight, dtype=np.float32)
    in_maps = [
        {"x": input_b[c * SHARD:(c + 1) * SHARD], "w": weight}
        for c in range(N_CORES)
    ]
    res = run_bass_kernel_spmd(nc, in_maps, list(range(N_CORES)), trace=trace)
    out = np.concatenate([res.results[c]["out"] for c in range(N_CORES)], axis=0)
    return out, res


def kernel(input_b, weight):
    out, _ = run_sharded(input_b, weight, trace=False)
    return out
